# revision 45
# baseline (speedup 1.0000x reference)
"""Trainium2 Bass kernel for cross-modal channel-attention fusion (CCDPA).

Math (per batch b):
  pooled[c,m,d] = mean_{w,h} x_m[b,c,d,w,h]
  q = Wq @ pooled[:,0,:] + bq ; k_m = Wk @ pooled[:,m,:] + bk
  a[c,m] = softmax_m(q[c]·k_m[c] / sqrt(D))
  out[b,o,s] = sum_m a[o,m] * (Wc[m] @ x_m[b,:,s] + bc[m,o])
             = sum_m (a[o,m]*Wc[m,o,:]) @ x_m[b,:,s]  + sum_m a[o,m]*bc[m,o]

Sharding/schedule: 8 cores, 4 pipelined phases (= the 4 batches). In phase t
every core reads ITS dpc=D/8 d-slices of batch t once (16 MiB fp32); the ACT
engine converts them to a bf16 SBUF cache and produces the pooling sums as a
free side effect (activation Copy with accum_out). The per-core pooled
partials are exchanged with one 8-way AllGather (16 KiB) launched purely by
DMA (SBUF DMA-transpose + scatter, no compute engine), attention weights are
computed on-device, the conv weights are a-scaled, and the batch's GEMMs run
from the bf16 cache (no second HBM read). HBM traffic is the floor: 64 MiB
read + 16 MiB written per core.

Emission is software-pipelined one phase deep: iteration t emits
[attention-tail(t-1), reads(t), AG-launch(t), GEMM(t-1)] so the in-order
engine streams never block the read pipeline on collective latency.

The 1/(W*H) pooling mean and the 1/sqrt(D) logit scale are folded into the
Wq/Wk weights host-side, and bq/bk ride along as an extra contraction row
(augmented [D+1, D] weight matrices against pooled-sum vectors with an
appended ones-row).
"""

from contextlib import ExitStack

import numpy as np

import concourse.bacc as bacc
import concourse.bass as bass
import concourse.mybir as mybir
import concourse.tile as tile
from concourse.bass_utils import run_bass_kernel_spmd

F32 = mybir.dt.float32
BF16 = mybir.dt.bfloat16

B, C, D, W, H = 4, 256, 32, 32, 32
NCORES = 8
M = 4  # modalities
CI = 2  # 128-row halves of C
T = B  # phases == batches
DPC = D // NCORES  # d-slices per core per phase
WH = W * H


def _emit_program(nc, wh=WH, dpc=DPC, stage=3):
    f32 = F32
    s = dpc * wh  # free elems per (m, ci) cache tile
    dd = dpc * NCORES  # full D for this (possibly scaled-down) config
    fr = dpc * CI * M  # praw free width
    chunk = min(s, 2048)  # staging DMA chunk (1 MiB at full size)
    nch = s // chunk
    dlpc = max(1, chunk // wh)  # d-slices per staging chunk
    AX = mybir.AxisListType.X
    AF = mybir.ActivationFunctionType

    xs = [
        nc.dram_tensor(f"x{m}", [T * C, s], f32, kind="ExternalInput")
        for m in range(M)
    ]
    wqT_d = nc.dram_tensor("wqTaug", [dd + 1, dd], f32, kind="ExternalInput")
    wkT_d = nc.dram_tensor("wkTaug", [dd + 1, dd], f32, kind="ExternalInput")
    # wcP[o, m*C + c] = Wc[m, o, c]
    wc_d = nc.dram_tensor("wcP", [C, M * C], f32, kind="ExternalInput")
    bct_d = nc.dram_tensor("bcT", [C, M], f32, kind="ExternalInput")
    id_d = nc.dram_tensor("ident", [128, 128], f32, kind="ExternalInput")
    out_d = nc.dram_tensor("out", [T * C, s], f32, kind="ExternalOutput")

    with tile.TileContext(nc) as tc, ExitStack() as ctx:
        const = ctx.enter_context(tc.tile_pool(name="const", bufs=1))
        stgp = ctx.enter_context(tc.tile_pool(name="stg", bufs=3))
        cachep = ctx.enter_context(tc.tile_pool(name="cache", bufs=2))
        outp = ctx.enter_context(tc.tile_pool(name="outp", bufs=2))
        attn = ctx.enter_context(tc.tile_pool(name="attn", bufs=2))
        wpool = ctx.enter_context(tc.tile_pool(name="wpool", bufs=1))
        scrp = ctx.enter_context(tc.tile_pool(name="scr", bufs=2))
        psA = ctx.enter_context(tc.tile_pool(name="psA", bufs=2, space="PSUM"))
        psM = ctx.enter_context(tc.tile_pool(name="psM", bufs=5, space="PSUM"))
        psJ = ctx.enter_context(tc.tile_pool(name="psJ", bufs=1, space="PSUM"))
        dramp = ctx.enter_context(tc.tile_pool(name="dramp", bufs=2, space="DRAM"))

        # ---- constants (off critical path) ----
        ident = const.tile([128, 128], f32, tag="ident", name="ident")
        nc.sync.dma_start(out=ident[:], in_=id_d[:])
        wqf = const.tile([dd + 1, dd], f32, tag="wqTf", name="wqTf")
        nc.sync.dma_start(out=wqf[:], in_=wqT_d[:])
        wq_sb = const.tile([dd + 1, dd], BF16, tag="wqT", name="wqT")
        nc.scalar.activation(wq_sb[:], wqf[:], mybir.ActivationFunctionType.Copy)
        wkf = const.tile([dd + 1, dd], f32, tag="wkTf", name="wkTf")
        nc.sync.dma_start(out=wkf[:], in_=wkT_d[:])
        wk_sb = const.tile([dd + 1, dd], BF16, tag="wkT", name="wkT")
        nc.scalar.activation(wk_sb[:], wkf[:], mybir.ActivationFunctionType.Copy)
        wc_sb = []
        for oi in range(CI):
            t_ = const.tile([128, M * C], f32, tag=f"wc{oi}", name=f"wc{oi}")
            nc.sync.dma_start(out=t_[:], in_=wc_d[oi * 128 : (oi + 1) * 128, :])
            wc_sb.append(t_)
        bct_sb = []
        for k in range(CI):
            t_ = const.tile([128, M], f32, tag=f"bct{k}", name=f"bct{k}")
            nc.sync.dma_start(out=t_[:], in_=bct_d[k * 128 : (k + 1) * 128, :])
            bct_sb.append(t_)

        nw = min(512, wh)  # PSUM bank limit: 512 fp32 per partition
        n_nh = wh // nw

        def emit_reads(t):
            """Stream phase t's x; ACT converts to bf16 cache with pooled
            sums accumulated as a side effect (accum_out)."""
            praw = attn.tile([128, fr], f32, tag="praw", name="praw")
            cache = {}
            unit = 0
            n_units = M * CI * nch * dlpc
            for m in range(M):
                for ci in range(CI):
                    ct = cachep.tile([128, s], BF16, tag=f"c{m}{ci}", name=f"c{m}{ci}")
                    cache[(m, ci)] = ct
                    for j in range(nch):
                        stg = stgp.tile([128, chunk], f32, tag="stg", name="stg")
                        nc.sync.dma_start(
                            out=stg[:],
                            in_=xs[m][
                                t * C + ci * 128 : t * C + (ci + 1) * 128,
                                j * chunk : (j + 1) * chunk,
                            ],
                        )
                        # keep-warm matmul, data-gated on this chunk, so the
                        # PE HAM clock gate stays open through the read phase
                        pj = psJ.tile([128, 256], f32, tag="pj", name="pj")
                        nc.tensor.matmul(
                            pj[:], lhsT=ident[:], rhs=stg[:, 0:256],
                            start=True, stop=True, skip_group_check=True,
                        )
                        for u in range(dlpc):
                            dl = j * dlpc + u
                            col = (dl * M + m) * CI + ci
                            dst = ct[:, dl * wh : (dl + 1) * wh]
                            src = stg[:, u * wh : (u + 1) * wh]
                            # split pooling+convert: ~2/3 on ACT (fused
                            # accum), ~1/3 as GPSIMD copy + DVE reduce
                            if unit % 3 != 2:
                                nc.scalar.activation(
                                    dst, src, AF.Copy,
                                    accum_out=praw[:, col : col + 1],
                                )
                            else:
                                nc.gpsimd.tensor_copy(dst, src)
                                nc.vector.reduce_sum(
                                    out=praw[:, col : col + 1], in_=src, axis=AX
                                )
                            unit += 1
            return praw, cache

        def emit_ag_launch(t, praw):
            """Pooled-sum exchange, no compute engines: SBUF DMA-transpose,
            scatter into cc_in, 8-way AllGather."""
            pst = psA.tile([fr, 128], f32, tag="att", name="att")
            nc.tensor.transpose(pst[:], praw[:], ident[:])
            trT = attn.tile([fr, 128], BF16, tag="trT", name="trT")
            nc.vector.tensor_copy(trT[:], pst[:])
            cc_in = dramp.tile([dpc, M * CI * 128], BF16, tag="cc_in", name="cc_in")
            cc_out = dramp.tile(
                [NCORES * dpc, M * CI * 128], BF16, tag="cc_out", name="cc_out"
            )
            nc.sync.dma_start(
                out=cc_in[:].rearrange("dl (m ci c) -> (dl m ci) c", m=M, ci=CI),
                in_=trT[:],
            )
            if stage < 3:  # debug: skip collective, fake gather with local data
                for h in range(NCORES):
                    nc.sync.dma_start(
                        out=cc_out[h * dpc : (h + 1) * dpc, :], in_=cc_in[:]
                    )
            else:
                nc.gpsimd.collective_compute(
                    "AllGather",
                    mybir.AluOpType.bypass,
                    replica_groups=[list(range(NCORES))],
                    ins=[cc_in.opt()],
                    outs=[cc_out.opt()],
                )
            return cc_out

        def emit_attn_tail(t, cc_out):
            """ptA load, q/k matmuls, softmax, a-scaled transposed weights."""
            # ptA[d, m*256 + ci*128 + c] = pooled_sum[c, m, d]; ones row at dd
            ptA = wpool.tile([dd + 1, M * CI * 128], BF16, tag="ptA", name="ptA")
            nc.vector.memset(ptA[:], 1.0)  # row dd stays 1.0 (bias ones-row)
            nc.scalar.dma_start(out=ptA[0:dd, :], in_=cc_out[:])

            a_sb, beff = [], []
            for k in range(CI):
                psq = psA.tile([128, dd], f32, tag="att", name="att")
                nc.tensor.matmul(
                    psq[:], lhsT=ptA[:, k * 128 : (k + 1) * 128], rhs=wq_sb[:],
                    start=True, stop=True,
                )
                q_sb = attn.tile([128, dd], f32, tag=f"q{k}", name=f"q{k}")
                nc.vector.tensor_copy(q_sb[:], psq[:])
                lg = attn.tile([128, M], f32, tag=f"lg{k}", name=f"lg{k}")
                for m in range(M):
                    psk = psA.tile([128, dd], f32, tag="att", name="att")
                    nc.tensor.matmul(
                        psk[:],
                        lhsT=ptA[:, m * C + k * 128 : m * C + (k + 1) * 128],
                        rhs=wk_sb[:],
                        start=True, stop=True,
                    )
                    scr = scrp.tile([128, dd], f32, tag="scr", name="scr")
                    nc.vector.tensor_mul(scr[:], q_sb[:], psk[:])
                    nc.vector.reduce_sum(out=lg[:, m : m + 1], in_=scr[:], axis=AX)
                mx = attn.tile([128, 1], f32, tag=f"mx{k}", name=f"mx{k}")
                nc.vector.reduce_max(out=mx[:], in_=lg[:], axis=AX)
                nc.vector.tensor_scalar_sub(out=lg[:], in0=lg[:], scalar1=mx[:])
                ex = attn.tile([128, M], f32, tag=f"ex{k}", name=f"ex{k}")
                nc.scalar.activation(ex[:], lg[:], AF.Exp)
                sm = attn.tile([128, 1], f32, tag=f"sm{k}", name=f"sm{k}")
                nc.vector.reduce_sum(out=sm[:], in_=ex[:], axis=AX)
                rc = attn.tile([128, 1], f32, tag=f"rc{k}", name=f"rc{k}")
                nc.vector.reciprocal(out=rc[:], in_=sm[:])
                at = attn.tile([128, M], f32, tag=f"a{k}", name=f"a{k}")
                nc.vector.tensor_scalar_mul(out=at[:], in0=ex[:], scalar1=rc[:])
                a_sb.append(at)
                scb = scrp.tile([128, M], f32, tag="scb", name="scb")
                be = attn.tile([128, 1], f32, tag=f"be{k}", name=f"be{k}")
                nc.vector.tensor_mul(scb[:], at[:], bct_sb[k][:])
                nc.vector.reduce_sum(out=be[:], in_=scb[:], axis=AX)
                beff.append(be)

            # weff[oi] = a[:,m] * wc rows; wt[ci] = weff^T (bf16)
            weff = [
                wpool.tile([128, M * C], f32, tag=f"weff{oi}", name=f"weff{oi}")
                for oi in range(CI)
            ]
            for oi in range(CI):
                for m in range(M):
                    nc.vector.tensor_scalar_mul(
                        out=weff[oi][:, m * C : (m + 1) * C],
                        in0=wc_sb[oi][:, m * C : (m + 1) * C],
                        scalar1=a_sb[oi][:, m : m + 1],
                    )
            wt = [
                wpool.tile([128, M * C], BF16, tag=f"wt{ci}", name=f"wt{ci}")
                for ci in range(CI)
            ]
            for m in range(M):
                for oi in range(CI):
                    for ci in range(CI):
                        psw = psA.tile([128, 128], f32, tag="att", name="att")
                        nc.tensor.transpose(
                            psw[:],
                            weff[oi][:, m * C + ci * 128 : m * C + (ci + 1) * 128],
                            ident[:],
                        )
                        nc.vector.tensor_copy(
                            wt[ci][:, m * C + oi * 128 : m * C + (oi + 1) * 128],
                            psw[:],
                        )
            return wt, beff

        def emit_gemm(t, cache, wt, beff):
            for dl in range(dpc):
                for oi in range(CI):
                    ot = outp.tile([128, wh], f32, tag="ot", name="ot")
                    for nh in range(n_nh):
                        ps = psM.tile([128, nw], f32, tag="ps", name="ps")
                        off = dl * wh + nh * nw
                        for m in range(M):
                            for ci in range(CI):
                                nc.tensor.matmul(
                                    ps[:],
                                    lhsT=wt[ci][
                                        :, m * C + oi * 128 : m * C + (oi + 1) * 128
                                    ],
                                    rhs=cache[(m, ci)][:, off : off + nw],
                                    start=(m == 0 and ci == 0),
                                    stop=(m == M - 1 and ci == CI - 1),
                                )
                        nc.vector.tensor_scalar_add(
                            out=ot[:, nh * nw : (nh + 1) * nw],
                            in0=ps[:],
                            scalar1=beff[oi][:],
                        )
                    nc.scalar.dma_start(
                        out=out_d[
                            t * C + oi * 128 : t * C + (oi + 1) * 128,
                            dl * wh : (dl + 1) * wh,
                        ],
                        in_=ot[:],
                    )

        pending = None  # (t, cache, cc_out) awaiting attention tail + GEMM
        for t in range(T):
            if pending is not None:
                wt, beff = emit_attn_tail(pending[0], pending[2])
            praw, cache = emit_reads(t)
            cc_out = emit_ag_launch(t, praw)
            if pending is not None:
                emit_gemm(pending[0], pending[1], wt, beff)
            pending = (t, cache, cc_out)
        wt, beff = emit_attn_tail(pending[0], pending[2])
        emit_gemm(pending[0], pending[1], wt, beff)
    return nc


_CACHED = {}
LAST_RESULTS = None


def _build(wh=WH, dpc=DPC, stage=3):
    key = (wh, dpc, stage)
    if key not in _CACHED:
        nc = bacc.Bacc(
            "TRN2",
            target_bir_lowering=False,
            debug=False,
            enable_asserts=False,
            num_devices=NCORES,
        )
        _emit_program(nc, wh=wh, dpc=dpc, stage=stage)
        nc.compile()
        _CACHED[key] = nc
    return _CACHED[key]


def _host_prep(Wq, bq, Wk, bk, Wc, bc, wh_pool, d):
    """Fold pooling mean + logit scale into augmented [D+1, D] q/k weights;
    permute Wc to [o, m*C + c]."""
    scale_q = 1.0 / (wh_pool * np.sqrt(np.float32(d)))
    wqTaug = np.concatenate(
        [(Wq * scale_q).T, (bq / np.sqrt(np.float32(d)))[None, :]], axis=0
    ).astype(np.float32)
    wkTaug = np.concatenate([(Wk / wh_pool).T, bk[None, :]], axis=0).astype(np.float32)
    wcP = np.ascontiguousarray(
        Wc.transpose(1, 0, 2).reshape(Wc.shape[1], -1)
    ).astype(np.float32)
    bcT = np.ascontiguousarray(bc.T).astype(np.float32)
    ident = np.eye(128, dtype=np.float32)
    return wqTaug, wkTaug, wcP, bcT, ident


def _make_in_maps(ms, Wq, bq, Wk, bk, Wc, bc, wh_pool, dpc):
    d = dpc * NCORES
    wqTaug, wkTaug, wcP, bcT, ident = _host_prep(Wq, bq, Wk, bk, Wc, bc, wh_pool, d)
    in_maps = []
    for p in range(NCORES):
        im = {
            f"x{m}": np.ascontiguousarray(
                ms[m][:, :, p * dpc : (p + 1) * dpc]
            ).reshape(T * C, dpc * wh_pool)
            for m in range(M)
        }
        im.update(wqTaug=wqTaug, wkTaug=wkTaug, wcP=wcP, bcT=bcT, ident=ident)
        in_maps.append(im)
    return in_maps


def kernel(m1, m2, m3, m4, Wq, bq, Wk, bk, Wc, bc, **run_kwargs):
    ms = [np.asarray(x, dtype=np.float32) for x in (m1, m2, m3, m4)]
    Wq, bq, Wk, bk, Wc, bc = (
        np.asarray(x, dtype=np.float32) for x in (Wq, bq, Wk, bk, Wc, bc)
    )
    nc = _build()
    in_maps = _make_in_maps(ms, Wq, bq, Wk, bk, Wc, bc, WH, DPC)
    global LAST_RESULTS
    res = run_bass_kernel_spmd(
        nc, in_maps, core_ids=list(range(NCORES)), **run_kwargs
    )
    LAST_RESULTS = res
    out = np.empty((B, C, D, W, H), np.float32)
    for p in range(NCORES):
        out[:, :, p * DPC : (p + 1) * DPC] = res.results[p]["out"].reshape(
            B, C, DPC, W, H
        )
    return out


# revision 46
# speedup vs baseline: 1.1360x; 1.1360x over previous
"""Trainium2 Bass kernel for cross-modal channel-attention fusion (CCDPA).

Math (per batch b):
  pooled[c,m,d] = mean_{w,h} x_m[b,c,d,w,h]
  q = Wq @ pooled[:,0,:] + bq ; k_m = Wk @ pooled[:,m,:] + bk
  a[c,m] = softmax_m(q[c]·k_m[c] / sqrt(D))
  out[b,o,s] = sum_m a[o,m] * (Wc[m] @ x_m[b,:,s] + bc[m,o])
             = sum_m (a[o,m]*Wc[m,o,:]) @ x_m[b,:,s]  + sum_m a[o,m]*bc[m,o]

Sharding/schedule: 8 cores, 4 pipelined phases (= the 4 batches). In phase t
every core reads ITS dpc=D/8 d-slices of batch t once (16 MiB fp32); the ACT
engine converts them to a bf16 SBUF cache and produces the pooling sums as a
free side effect (activation Copy with accum_out). The per-core pooled
partials are exchanged with one 8-way AllGather (16 KiB) launched purely by
DMA (SBUF DMA-transpose + scatter, no compute engine), attention weights are
computed on-device, the conv weights are a-scaled, and the batch's GEMMs run
from the bf16 cache (no second HBM read). HBM traffic is the floor: 64 MiB
read + 16 MiB written per core.

Emission is software-pipelined one phase deep: iteration t emits
[attention-tail(t-1), reads(t), AG-launch(t), GEMM(t-1)] so the in-order
engine streams never block the read pipeline on collective latency.

The 1/(W*H) pooling mean and the 1/sqrt(D) logit scale are folded into the
Wq/Wk weights host-side, and bq/bk ride along as an extra contraction row
(augmented [D+1, D] weight matrices against pooled-sum vectors with an
appended ones-row).
"""

from contextlib import ExitStack

import numpy as np

import concourse.bacc as bacc
import concourse.bass as bass
import concourse.mybir as mybir
import concourse.tile as tile
from concourse.bass_utils import run_bass_kernel_spmd

F32 = mybir.dt.float32
BF16 = mybir.dt.bfloat16

B, C, D, W, H = 4, 256, 32, 32, 32
NCORES = 8
M = 4  # modalities
CI = 2  # 128-row halves of C
T = B  # phases == batches
DPC = D // NCORES  # d-slices per core per phase
WH = W * H


def _emit_program(nc, wh=WH, dpc=DPC, stage=3):
    f32 = F32
    s = dpc * wh  # free elems per (m, ci) cache tile
    dd = dpc * NCORES  # full D for this (possibly scaled-down) config
    fr = dpc * CI * M  # praw free width
    chunk = min(s, 2048)  # staging DMA chunk (1 MiB at full size)
    nch = s // chunk
    dlpc = max(1, chunk // wh)  # d-slices per staging chunk
    AX = mybir.AxisListType.X
    AF = mybir.ActivationFunctionType

    xs = [
        nc.dram_tensor(f"x{m}", [T * C, s], f32, kind="ExternalInput")
        for m in range(M)
    ]
    wqT_d = nc.dram_tensor("wqTaug", [dd + 1, dd], f32, kind="ExternalInput")
    wkT_d = nc.dram_tensor("wkTaug", [dd + 1, dd], f32, kind="ExternalInput")
    # wcP[o, m*C + c] = Wc[m, o, c]
    wc_d = nc.dram_tensor("wcP", [C, M * C], f32, kind="ExternalInput")
    bct_d = nc.dram_tensor("bcT", [C, M], f32, kind="ExternalInput")
    id_d = nc.dram_tensor("ident", [128, 128], f32, kind="ExternalInput")
    out_d = nc.dram_tensor("out", [T * C, s], f32, kind="ExternalOutput")

    with tile.TileContext(nc) as tc, ExitStack() as ctx:
        const = ctx.enter_context(tc.tile_pool(name="const", bufs=1))
        stgp = ctx.enter_context(tc.tile_pool(name="stg", bufs=3))
        cachep = ctx.enter_context(tc.tile_pool(name="cache", bufs=2))
        outp = ctx.enter_context(tc.tile_pool(name="outp", bufs=2))
        attn = ctx.enter_context(tc.tile_pool(name="attn", bufs=2))
        wpool = ctx.enter_context(tc.tile_pool(name="wpool", bufs=1))
        scrp = ctx.enter_context(tc.tile_pool(name="scr", bufs=2))
        psA = ctx.enter_context(tc.tile_pool(name="psA", bufs=2, space="PSUM"))
        psM = ctx.enter_context(tc.tile_pool(name="psM", bufs=5, space="PSUM"))
        psJ = ctx.enter_context(tc.tile_pool(name="psJ", bufs=1, space="PSUM"))
        dramp = ctx.enter_context(tc.tile_pool(name="dramp", bufs=2, space="DRAM"))

        # ---- constants (off critical path) ----
        ident = const.tile([128, 128], f32, tag="ident", name="ident")
        nc.sync.dma_start(out=ident[:], in_=id_d[:])
        wqf = const.tile([dd + 1, dd], f32, tag="wqTf", name="wqTf")
        nc.sync.dma_start(out=wqf[:], in_=wqT_d[:])
        wq_sb = const.tile([dd + 1, dd], BF16, tag="wqT", name="wqT")
        nc.scalar.activation(wq_sb[:], wqf[:], mybir.ActivationFunctionType.Copy)
        wkf = const.tile([dd + 1, dd], f32, tag="wkTf", name="wkTf")
        nc.sync.dma_start(out=wkf[:], in_=wkT_d[:])
        wk_sb = const.tile([dd + 1, dd], BF16, tag="wkT", name="wkT")
        nc.scalar.activation(wk_sb[:], wkf[:], mybir.ActivationFunctionType.Copy)
        wc_sb = []
        for oi in range(CI):
            t_ = const.tile([128, M * C], f32, tag=f"wc{oi}", name=f"wc{oi}")
            nc.sync.dma_start(out=t_[:], in_=wc_d[oi * 128 : (oi + 1) * 128, :])
            wc_sb.append(t_)
        bct_sb = []
        for k in range(CI):
            t_ = const.tile([128, M], f32, tag=f"bct{k}", name=f"bct{k}")
            nc.sync.dma_start(out=t_[:], in_=bct_d[k * 128 : (k + 1) * 128, :])
            bct_sb.append(t_)

        nw = min(512, wh)  # PSUM bank limit: 512 fp32 per partition
        n_nh = wh // nw

        def emit_reads(t):
            """Stream phase t's x; ACT converts to bf16 cache with pooled
            sums accumulated as a side effect (accum_out)."""
            praw = attn.tile([128, fr], f32, tag="praw", name="praw")
            cache = {}
            unit = 0
            n_units = M * CI * nch * dlpc
            for m in range(M):
                for ci in range(CI):
                    ct = cachep.tile([128, s], BF16, tag=f"c{m}{ci}", name=f"c{m}{ci}")
                    cache[(m, ci)] = ct
                    for j in range(nch):
                        stg = stgp.tile([128, chunk], f32, tag="stg", name="stg")
                        nc.sync.dma_start(
                            out=stg[:],
                            in_=xs[m][
                                t * C + ci * 128 : t * C + (ci + 1) * 128,
                                j * chunk : (j + 1) * chunk,
                            ],
                        )
                        for u in range(dlpc):
                            dl = j * dlpc + u
                            col = (dl * M + m) * CI + ci
                            dst = ct[:, dl * wh : (dl + 1) * wh]
                            src = stg[:, u * wh : (u + 1) * wh]
                            # split pooling+convert: 3/4 on ACT (fused
                            # accum), 1/4 as DVE copy + reduce
                            if unit % 4 != 3:
                                nc.scalar.activation(
                                    dst, src, AF.Copy,
                                    accum_out=praw[:, col : col + 1],
                                )
                            else:
                                nc.vector.tensor_copy(dst, src)
                                nc.vector.reduce_sum(
                                    out=praw[:, col : col + 1], in_=src, axis=AX
                                )
                            unit += 1
            return praw, cache

        def emit_ag_launch(t, praw):
            """Pooled-sum exchange, no compute engines: SBUF DMA-transpose,
            scatter into cc_in, 8-way AllGather."""
            pst = psA.tile([fr, 128], f32, tag="att", name="att")
            nc.tensor.transpose(pst[:], praw[:], ident[:])
            trT = attn.tile([fr, 128], BF16, tag="trT", name="trT")
            nc.vector.tensor_copy(trT[:], pst[:])
            cc_in = dramp.tile([dpc, M * CI * 128], BF16, tag="cc_in", name="cc_in")
            cc_out = dramp.tile(
                [NCORES * dpc, M * CI * 128], BF16, tag="cc_out", name="cc_out"
            )
            nc.sync.dma_start(
                out=cc_in[:].rearrange("dl (m ci c) -> (dl m ci) c", m=M, ci=CI),
                in_=trT[:],
            )
            if stage < 3:  # debug: skip collective, fake gather with local data
                for h in range(NCORES):
                    nc.sync.dma_start(
                        out=cc_out[h * dpc : (h + 1) * dpc, :], in_=cc_in[:]
                    )
            else:
                nc.gpsimd.collective_compute(
                    "AllGather",
                    mybir.AluOpType.bypass,
                    replica_groups=[list(range(NCORES))],
                    ins=[cc_in.opt()],
                    outs=[cc_out.opt()],
                )
            return cc_out

        def emit_attn_tail(t, cc_out):
            """ptA load, q/k matmuls, softmax, a-scaled transposed weights."""
            # ptA[d, m*256 + ci*128 + c] = pooled_sum[c, m, d]; ones row at dd
            ptA = wpool.tile([dd + 1, M * CI * 128], BF16, tag="ptA", name="ptA")
            nc.vector.memset(ptA[:], 1.0)  # row dd stays 1.0 (bias ones-row)
            nc.scalar.dma_start(out=ptA[0:dd, :], in_=cc_out[:])

            a_sb, beff = [], []
            for k in range(CI):
                psq = psA.tile([128, dd], f32, tag="att", name="att")
                nc.tensor.matmul(
                    psq[:], lhsT=ptA[:, k * 128 : (k + 1) * 128], rhs=wq_sb[:],
                    start=True, stop=True,
                )
                q_sb = attn.tile([128, dd], f32, tag=f"q{k}", name=f"q{k}")
                nc.vector.tensor_copy(q_sb[:], psq[:])
                lg = attn.tile([128, M], f32, tag=f"lg{k}", name=f"lg{k}")
                for m in range(M):
                    psk = psA.tile([128, dd], f32, tag="att", name="att")
                    nc.tensor.matmul(
                        psk[:],
                        lhsT=ptA[:, m * C + k * 128 : m * C + (k + 1) * 128],
                        rhs=wk_sb[:],
                        start=True, stop=True,
                    )
                    scr = scrp.tile([128, dd], f32, tag="scr", name="scr")
                    nc.vector.tensor_mul(scr[:], q_sb[:], psk[:])
                    nc.vector.reduce_sum(out=lg[:, m : m + 1], in_=scr[:], axis=AX)
                mx = attn.tile([128, 1], f32, tag=f"mx{k}", name=f"mx{k}")
                nc.vector.reduce_max(out=mx[:], in_=lg[:], axis=AX)
                nc.vector.tensor_scalar_sub(out=lg[:], in0=lg[:], scalar1=mx[:])
                ex = attn.tile([128, M], f32, tag=f"ex{k}", name=f"ex{k}")
                nc.scalar.activation(ex[:], lg[:], AF.Exp)
                sm = attn.tile([128, 1], f32, tag=f"sm{k}", name=f"sm{k}")
                nc.vector.reduce_sum(out=sm[:], in_=ex[:], axis=AX)
                rc = attn.tile([128, 1], f32, tag=f"rc{k}", name=f"rc{k}")
                nc.vector.reciprocal(out=rc[:], in_=sm[:])
                at = attn.tile([128, M], f32, tag=f"a{k}", name=f"a{k}")
                nc.vector.tensor_scalar_mul(out=at[:], in0=ex[:], scalar1=rc[:])
                a_sb.append(at)
                scb = scrp.tile([128, M], f32, tag="scb", name="scb")
                be = attn.tile([128, 1], f32, tag=f"be{k}", name=f"be{k}")
                nc.vector.tensor_mul(scb[:], at[:], bct_sb[k][:])
                nc.vector.reduce_sum(out=be[:], in_=scb[:], axis=AX)
                beff.append(be)

            # weff[oi] = a[:,m] * wc rows; wt[ci] = weff^T (bf16)
            weff = [
                wpool.tile([128, M * C], f32, tag=f"weff{oi}", name=f"weff{oi}")
                for oi in range(CI)
            ]
            for oi in range(CI):
                for m in range(M):
                    nc.vector.tensor_scalar_mul(
                        out=weff[oi][:, m * C : (m + 1) * C],
                        in0=wc_sb[oi][:, m * C : (m + 1) * C],
                        scalar1=a_sb[oi][:, m : m + 1],
                    )
            wt = [
                wpool.tile([128, M * C], BF16, tag=f"wt{ci}", name=f"wt{ci}")
                for ci in range(CI)
            ]
            for m in range(M):
                for oi in range(CI):
                    for ci in range(CI):
                        psw = psA.tile([128, 128], f32, tag="att", name="att")
                        nc.tensor.transpose(
                            psw[:],
                            weff[oi][:, m * C + ci * 128 : m * C + (ci + 1) * 128],
                            ident[:],
                        )
                        nc.vector.tensor_copy(
                            wt[ci][:, m * C + oi * 128 : m * C + (oi + 1) * 128],
                            psw[:],
                        )
            return wt, beff

        def emit_gemm(t, cache, wt, beff):
            for dl in range(dpc):
                for oi in range(CI):
                    ot = outp.tile([128, wh], f32, tag="ot", name="ot")
                    for nh in range(n_nh):
                        ps = psM.tile([128, nw], f32, tag="ps", name="ps")
                        off = dl * wh + nh * nw
                        for m in range(M):
                            for ci in range(CI):
                                nc.tensor.matmul(
                                    ps[:],
                                    lhsT=wt[ci][
                                        :, m * C + oi * 128 : m * C + (oi + 1) * 128
                                    ],
                                    rhs=cache[(m, ci)][:, off : off + nw],
                                    start=(m == 0 and ci == 0),
                                    stop=(m == M - 1 and ci == CI - 1),
                                )
                        nc.vector.tensor_scalar_add(
                            out=ot[:, nh * nw : (nh + 1) * nw],
                            in0=ps[:],
                            scalar1=beff[oi][:],
                        )
                    nc.scalar.dma_start(
                        out=out_d[
                            t * C + oi * 128 : t * C + (oi + 1) * 128,
                            dl * wh : (dl + 1) * wh,
                        ],
                        in_=ot[:],
                    )

        pending = None  # (t, cache, cc_out) awaiting attention tail + GEMM
        for t in range(T):
            if pending is not None:
                wt, beff = emit_attn_tail(pending[0], pending[2])
            praw, cache = emit_reads(t)
            cc_out = emit_ag_launch(t, praw)
            if pending is not None:
                emit_gemm(pending[0], pending[1], wt, beff)
            pending = (t, cache, cc_out)
        wt, beff = emit_attn_tail(pending[0], pending[2])
        emit_gemm(pending[0], pending[1], wt, beff)
    return nc


_CACHED = {}
LAST_RESULTS = None


def _build(wh=WH, dpc=DPC, stage=3):
    key = (wh, dpc, stage)
    if key not in _CACHED:
        nc = bacc.Bacc(
            "TRN2",
            target_bir_lowering=False,
            debug=False,
            enable_asserts=False,
            num_devices=NCORES,
        )
        _emit_program(nc, wh=wh, dpc=dpc, stage=stage)
        nc.compile()
        _CACHED[key] = nc
    return _CACHED[key]


def _host_prep(Wq, bq, Wk, bk, Wc, bc, wh_pool, d):
    """Fold pooling mean + logit scale into augmented [D+1, D] q/k weights;
    permute Wc to [o, m*C + c]."""
    scale_q = 1.0 / (wh_pool * np.sqrt(np.float32(d)))
    wqTaug = np.concatenate(
        [(Wq * scale_q).T, (bq / np.sqrt(np.float32(d)))[None, :]], axis=0
    ).astype(np.float32)
    wkTaug = np.concatenate([(Wk / wh_pool).T, bk[None, :]], axis=0).astype(np.float32)
    wcP = np.ascontiguousarray(
        Wc.transpose(1, 0, 2).reshape(Wc.shape[1], -1)
    ).astype(np.float32)
    bcT = np.ascontiguousarray(bc.T).astype(np.float32)
    ident = np.eye(128, dtype=np.float32)
    return wqTaug, wkTaug, wcP, bcT, ident


def _make_in_maps(ms, Wq, bq, Wk, bk, Wc, bc, wh_pool, dpc):
    d = dpc * NCORES
    wqTaug, wkTaug, wcP, bcT, ident = _host_prep(Wq, bq, Wk, bk, Wc, bc, wh_pool, d)
    in_maps = []
    for p in range(NCORES):
        im = {
            f"x{m}": np.ascontiguousarray(
                ms[m][:, :, p * dpc : (p + 1) * dpc]
            ).reshape(T * C, dpc * wh_pool)
            for m in range(M)
        }
        im.update(wqTaug=wqTaug, wkTaug=wkTaug, wcP=wcP, bcT=bcT, ident=ident)
        in_maps.append(im)
    return in_maps


def kernel(m1, m2, m3, m4, Wq, bq, Wk, bk, Wc, bc, **run_kwargs):
    ms = [np.asarray(x, dtype=np.float32) for x in (m1, m2, m3, m4)]
    Wq, bq, Wk, bk, Wc, bc = (
        np.asarray(x, dtype=np.float32) for x in (Wq, bq, Wk, bk, Wc, bc)
    )
    nc = _build()
    in_maps = _make_in_maps(ms, Wq, bq, Wk, bk, Wc, bc, WH, DPC)
    global LAST_RESULTS
    res = run_bass_kernel_spmd(
        nc, in_maps, core_ids=list(range(NCORES)), **run_kwargs
    )
    LAST_RESULTS = res
    out = np.empty((B, C, D, W, H), np.float32)
    for p in range(NCORES):
        out[:, :, p * DPC : (p + 1) * DPC] = res.results[p]["out"].reshape(
            B, C, DPC, W, H
        )
    return out
